# revision 60
# baseline (speedup 1.0000x reference)
"""3-layer GraphSAGE (mean aggr) on Trainium2, 8-core SPMD.

Strategy (graph/data parallel, ReduceScatter formulation; cost-model
time ~191us vs the 334us AllGather baseline):
  - Nodes padded 10000 -> 10240; degree-balancing permutation assigns
    them to 80 blocks of 128 (block in-degree ~2000). Core r owns blocks
    [10r, 10r+10); one SPMD program runs on all 8 cores.
  - Neighbor aggregation everywhere is "gather rows + 0/1 one-hot
    matmul" with PSUM accumulation. One-hot caches are HOST-built (pure
    graph preprocessing) and DMA'd in: slots are deduplicated per
    (source, dst-block) pair, so a repeated source contributes one
    gathered row whose one-hot row carries the edge multiplicities.
    GPSIMD dma_gather throughput in the cost model is per-row-element
    (~0.15 + 0.0066*elems ns/row), so x (128 cols) and t3 (64 cols
    padded to 128) gather at ~1ns/row and h1 (256 cols) at ~1.8ns/row.
  - L1 (x replicated): per own dst block, gather x[src], one-hot matmul
    to node-major sums, 1/deg scale (DVE), PE transpose via identity
    matmul -> mean1^T, then h1 = relu(mean1@w1l + x@w1r + b1) stored
    node-major (DRAM, gather source for L2) and transposed
    (SBUF-resident, L2 root term). The block loop is software-pipelined
    one block ahead so the in-order PE queue never stalls on DVE hops.
  - L2/L3 aggregation is SOURCE-partitioned: each core computes partial
    neighbor sums for ALL 80 dst blocks from its OWN h1/t3 rows (local
    gathers), written block-major to DRAM; one ReduceScatter(add) then
    delivers each core the summed rows it owns. RS cost is
    15us + out_bytes/40GBps, so RS moves 327KB(f8)+160KB instead of the
    AllGather baseline's 5.2MB+2.6MB: collective time 227us -> 42us.
    The L2 exchange runs in fp8 (partials + reduced output; adds ~8e-3
    rel err, well inside the 2e-2 budget); the L3 logits exchange stays
    fp16. 1/deg is applied after the RS, folded into the PE transpose
    as a diagonal-matrix matmul (host-built diag of 1/deg).
  - L2 own blocks are processed in PAIRS (256-wide free dim): h2^T
    slices accumulate 4 matmuls (w2l x mean^T + w2r x h1^T), relu+bias
    on DVE; t3 = h2@[w3l|w3r] in the same loop, t3 node-major to DRAM.
  - L3: same slot structure as L2 (one-hot cache + gather indices
    reused), partial sums staged 8 blocks per DMA -> RS -> own rows;
    out = sums/deg + r3 fused in one scalar_tensor_tensor; log_softmax
    with a single deferred Ln over all blocks' exp-sums (2 ACT table
    loads total); one batched output DMA.
  - PSUM (8 banks) is split into four shape-class rings shared by
    time-disjoint phases; relu/bias runs on DVE (tensor_scalar ADD,MAX)
    keeping ACT off the critical path except Exp/Ln.
"""

import numpy as np
BF = np.float16

N_NODES = 10000
NPAD = 10240
NCORES = 8
P = 128
NB = 10                      # own dst blocks per core
NBLK = NPAD // P             # 80 global blocks
PER_CORE = NB * P            # 1280
D_IN, D_H1, D_H2, D_OUT = 128, 256, 1024, 64
GSZ2 = 8                     # chunks per L2 partial gather group
GSZ3 = 16                    # chunks per L3 partial gather group
AHEAD = 16                   # chunk lookahead for gather emission

_CACHE = {}
LAST_RESULTS = None          # test harness reads exec_time_ns from here


def _build(MC1, C2):
    import os
    import concourse.bacc as bacc
    import concourse.mybir as mybir
    import concourse.tile as tile

    abl = set(os.environ.get("KABL", "").split(","))

    f32 = mybir.dt.float32
    bf16 = mybir.dt.float16
    f8 = mybir.dt.float8e4
    i16 = mybir.dt.int16
    nc = bacc.Bacc("TRN2", target_bir_lowering=False, debug=False,
                   num_devices=NCORES)
    def act_reduce_scatter(in_ap, out_ap):
        # collective issued off the Pool engine so it does not block the
        # gather stream
        nc.has_collectives = True
        eng = nc.gpsimd
        return eng.add_instruction(mybir.InstCollectiveCompute(
            name=f"I-{nc.next_id()}",
            kind="ReduceScatter", op=mybir.AluOpType.add,
            replica_groups=[list(range(NCORES))],
            ins=[eng.lower_ap(in_ap)], outs=[eng.lower_ap(out_ap)],
            unique_tensors="No", cc_dim="Partition"))

    NCH2 = int(sum(C2))
    cstart = [0]
    for c in C2:
        cstart.append(cstart[-1] + int(c))

    # gather groups over the L2/L3 chunk stream
    def mk_groups(gsz):
        groups = []
        lo = 0
        while lo < NCH2:
            n = min(gsz, NCH2 - lo)
            groups.append((lo, n))
            lo += n
        cg = np.zeros(NCH2, np.int64)
        co = np.zeros(NCH2, np.int64)
        for gi, (glo, gn) in enumerate(groups):
            cg[glo:glo + gn] = gi
            co[glo:glo + gn] = np.arange(gn)
        return groups, cg, co

    groups2, cgrp2, coff2 = mk_groups(GSZ2)
    groups3, cgrp3, coff3 = mk_groups(GSZ3)

    xbf = nc.dram_tensor("xbf", [NPAD, D_IN], bf16, kind="ExternalInput")
    xownT = nc.dram_tensor("xownT", [P, PER_CORE], bf16, kind="ExternalInput")
    w1l = nc.dram_tensor("w1l", [D_IN, D_H1], bf16, kind="ExternalInput")
    w1r = nc.dram_tensor("w1r", [D_IN, D_H1], bf16, kind="ExternalInput")
    b1 = nc.dram_tensor("b1", [1, D_H1], bf16, kind="ExternalInput")
    b1t = nc.dram_tensor("b1t", [P, 2], f32, kind="ExternalInput")
    w2l = nc.dram_tensor("w2l", [D_H1, D_H2], bf16, kind="ExternalInput")
    w2r = nc.dram_tensor("w2r", [D_H1, D_H2], bf16, kind="ExternalInput")
    b2t = nc.dram_tensor("b2t", [P, 8], f32, kind="ExternalInput")
    w3lr = nc.dram_tensor("w3lr", [D_H2, P], bf16, kind="ExternalInput")
    b3pad = nc.dram_tensor("b3pad", [1, P], bf16, kind="ExternalInput")
    gidx1 = nc.dram_tensor("gidx1", [P, NB * MC1 * 8], i16, kind="ExternalInput")
    gidx2 = nc.dram_tensor("gidx2", [P, NCH2 * 8], i16, kind="ExternalInput")
    oh1_in = nc.dram_tensor("oh1_in", [P, NB * MC1 * P], bf16,
                            kind="ExternalInput")
    oh2_in = nc.dram_tensor("oh2_in", [P, NCH2 * P], bf16,
                            kind="ExternalInput")
    invd_own = nc.dram_tensor("invd_own", [P, NB], f32, kind="ExternalInput")
    ident_in = nc.dram_tensor("ident_in", [P, P], bf16, kind="ExternalInput")
    diag_in = nc.dram_tensor("diag_in", [P, NB * P], bf16,
                             kind="ExternalInput")
    outp = nc.dram_tensor("out", [PER_CORE, D_OUT], f32, kind="ExternalOutput")

    EXP = mybir.ActivationFunctionType.Exp
    LN = mybir.ActivationFunctionType.Ln
    EQ = mybir.AluOpType.is_equal
    MUL = mybir.AluOpType.mult
    SUB = mybir.AluOpType.subtract
    ADD = mybir.AluOpType.add
    MAX = mybir.AluOpType.max
    AXX = mybir.AxisListType.X

    with tile.TileContext(nc) as tc:
        with (
            tc.tile_pool(name="const", bufs=1) as cp,
            tc.tile_pool(name="g1", bufs=4) as gp1,
            tc.tile_pool(name="g2", bufs=4) as gp2,
            tc.tile_pool(name="g3", bufs=3) as gp3,
            tc.tile_pool(name="small", bufs=6) as smp,
            tc.tile_pool(name="stage", bufs=2) as stp,
            tc.tile_pool(name="hts", bufs=2) as htp,
            tc.tile_pool(name="psA2", bufs=2, space="PSUM") as psA,
            tc.tile_pool(name="psO", bufs=2, space="PSUM") as psO,
            tc.tile_pool(name="psT", bufs=2, space="PSUM") as psT,
            tc.tile_pool(name="dram", bufs=1, space="DRAM") as dram,
        ):
            # ---- constants ----
            gidx1_sb = cp.tile([P, NB * MC1 * 8], i16, tag="gidx1")
            nc.sync.dma_start(gidx1_sb[:], gidx1[:])
            ones_t = cp.tile([1, P], bf16, tag="ones")
            nc.vector.memset(ones_t[:], 1.0)
            ident = cp.tile([P, P], bf16, tag="ident")
            nc.sync.dma_start(ident[:], ident_in[:])
            diag = cp.tile([P, NB, P], bf16, tag="diag")
            nc.sync.dma_start(diag[:], diag_in[:])
            gidx2_sb = cp.tile([P, NCH2 * 8], i16, tag="gidx2")
            nc.sync.dma_start(gidx2_sb[:], gidx2[:])
            invd_sb = cp.tile([P, NB], f32, tag="invd")
            nc.sync.dma_start(invd_sb[:], invd_own[:])
            w1l_sb = cp.tile([P, D_H1], bf16, tag="w1l")
            nc.sync.dma_start(w1l_sb[:], w1l[:])
            w1r_sb = cp.tile([P, D_H1], bf16, tag="w1r")
            nc.sync.dma_start(w1r_sb[:], w1r[:])
            b1_sb = cp.tile([1, D_H1], bf16, tag="b1")
            nc.sync.dma_start(b1_sb[:], b1[:])
            b1t_sb = cp.tile([P, 2], f32, tag="b1t")
            nc.sync.dma_start(b1t_sb[:], b1t[:])
            xT_res = cp.tile([P, PER_CORE], bf16, tag="xT")
            nc.sync.dma_start(xT_res[:], xownT[:])

            # one-hot caches: host-built, DMA'd in block/group sized slices
            ohc1 = cp.tile([P, NB * MC1, P], bf16, tag="ohc1")
            for b in range(NB):
                sl = slice(b * MC1 * P, (b + 1) * MC1 * P)
                nc.sync.dma_start(
                    ohc1[:, b * MC1:(b + 1) * MC1, :],
                    oh1_in[:, sl])
            ohc2 = cp.tile([P, NCH2, P], bf16, tag="ohc2")

            # resident cross-phase SBUF state
            h1T_res = cp.tile([P, 2, PER_CORE], bf16, tag="h1T")
            r3_res = cp.tile([P, NB, D_OUT], f32, tag="r3")
            ob_res = cp.tile([P, NB, D_OUT], f32, tag="ob")
            obf_res = cp.tile([P, NB, D_OUT], f32, tag="obf")
            ssum_res = cp.tile([P, NB], f32, tag="ssum")
            ls_res = cp.tile([P, NB], f32, tag="ls")

            # ---- DRAM intermediates ----
            h1_own = dram.tile([PER_CORE, D_H1], bf16, tag="h1o")
            t3_own = dram.tile([PER_CORE, P], bf16, tag="t3o")
            p2 = dram.tile([NBLK, P, D_H1], f8, tag="p2")
            m2 = dram.tile([NB, P, D_H1], f8, tag="m2")
            p3 = dram.tile([NBLK, P, D_OUT], bf16, tag="p3")
            m3 = dram.tile([NB, P, D_OUT], bf16, tag="m3")

            # ================= Layer 1 (own dst blocks) =================
            # one-block software pipeline: aggregate block b, then transform
            # block b-1, so the PE queue never stalls on ACT/DVE chain hops.
            aggs = {}

            def l1_transform(b):
                agg = aggs.pop(b)
                m1s = smp.tile([P, D_IN], bf16, tag="m1s")
                nc.vector.tensor_scalar(m1s[:], agg[:],
                                        invd_sb[:, b:b + 1], None, MUL)
                tp1 = psT.tile([P, P], f32, tag="tp")
                nc.tensor.matmul(tp1[:], m1s[:], ident[:], start=True,
                                 stop=True)
                meanT = smp.tile([P, D_IN], bf16, tag="meanT1")
                nc.scalar.copy(meanT[:], tp1[:])

                xT = xT_res[:, b * P:(b + 1) * P]
                op2 = psA.tile([P, 2, D_H1], f32, tag="agg2")
                op = op2[:, 0, :]
                nc.tensor.matmul(op, meanT[:], w1l_sb[:],
                                 start=True, stop=False)
                nc.tensor.matmul(op, xT, w1r_sb[:], start=False, stop=False)
                nc.tensor.matmul(op, ones_t[:], b1_sb[:],
                                 start=False, stop=True)
                h1blk = smp.tile([P, D_H1], bf16, tag="h1blk")
                nc.vector.tensor_scalar(h1blk[:], op, 0.0, None, MAX)
                nc.sync.dma_start(h1_own[b * P:(b + 1) * P, :], h1blk[:])
                tp2 = psO.tile([P, D_H1], f32, tag="outp")
                for s in range(2):
                    sl = slice(s * P, (s + 1) * P)
                    nc.tensor.matmul(tp2[:, sl], w1l_sb[:, sl],
                                     meanT[:], start=True, stop=False)
                    nc.tensor.matmul(tp2[:, sl], w1r_sb[:, sl],
                                     xT, start=False, stop=True)
                for s in range(2):
                    nc.vector.tensor_scalar(h1T_res[:, s, b * P:(b + 1) * P],
                                            tp2[:, s * P:(s + 1) * P],
                                            b1t_sb[:, s:s + 1], 0.0,
                                            ADD, MAX)

            NB1 = NB if "l1" not in abl else 0
            for b in range(NB1):
                gath = gp1.tile([P, MC1, D_IN], bf16, tag="gath1")
                h1c = MC1 // 2 + 1
                pieces = ((0, 4), (4, 4), (8, MC1 - 8)) if b == 0 else \
                    ((0, h1c), (h1c, MC1 - h1c))
                for g0, gn in pieces:
                    c0 = (b * MC1 + g0) * 8
                    nc.gpsimd.dma_gather(
                        gath[:, g0:g0 + gn, :], xbf[:],
                        gidx1_sb[:, c0:c0 + gn * 8],
                        gn * P, gn * P, D_IN, single_packet=False)
                agg = psA.tile([P, D_IN], f32, tag="agg1")
                for c in range(MC1):
                    nc.tensor.matmul(agg[:], ohc1[:, b * MC1 + c, :],
                                     gath[:, c, :],
                                     start=(c == 0), stop=(c == MC1 - 1))
                aggs[b] = agg
                if b > 0:
                    l1_transform(b - 1)
            if NB1:
                l1_transform(NB1 - 1)

            # layer-2/3 weights first needed after L1
            w2l_sb = cp.tile([P, 2, D_H2], bf16, tag="w2l")
            nc.sync.dma_start(w2l_sb[:], w2l.rearrange("(s p) n -> p s n", p=P))
            w2r_sb = cp.tile([P, 2, D_H2], bf16, tag="w2r")
            nc.sync.dma_start(w2r_sb[:], w2r.rearrange("(s p) n -> p s n", p=P))
            b2t_sb = cp.tile([P, 8], f32, tag="b2t")
            nc.sync.dma_start(b2t_sb[:], b2t[:])
            w3lr_sb = cp.tile([P, 8, P], bf16, tag="w3lr")
            nc.sync.dma_start(w3lr_sb[:], w3lr.rearrange("(s p) n -> p s n", p=P))
            b3_sb = cp.tile([1, P], bf16, tag="b3")
            nc.sync.dma_start(b3_sb[:], b3pad[:])

            # ========== Layer 2 partial sums for ALL dst blocks ==========
            state = {"next": 0, "tiles": {}}

            def need_chunks(pool, src_ap, elem, tag, target, groups, width,
                            load_oh=False):
                while (state["next"] < len(groups)
                       and groups[state["next"]][0] < target):
                    glo, gn = groups[state["next"]]
                    if load_oh:
                        nc.sync.dma_start(
                            ohc2[:, glo:glo + gn, :],
                            oh2_in[:, glo * P:(glo + gn) * P])
                    t = pool.tile([P, width, elem], bf16, tag=tag)
                    nc.gpsimd.dma_gather(
                        t[:, 0:gn, :], src_ap,
                        gidx2_sb[:, glo * 8:(glo + gn) * 8],
                        gn * P, gn * P, elem, single_packet=False)
                    state["tiles"][state["next"]] = t
                    state["next"] += 1

            if "l2p" not in abl:
                state["next"] = 0
                state["tiles"] = {}
                agg2 = None
                for g in range(NBLK):
                    c0, cn = cstart[g], int(C2[g])
                    need_chunks(gp2, h1_own[:], D_H1, "gath2",
                                min(c0 + cn + 24, NCH2), groups2, GSZ2,
                                load_oh=True)
                    q = g % 2
                    if q == 0:
                        agg2 = psA.tile([P, 2, D_H1], f32, tag="agg2")
                    for ci in range(cn):
                        c = c0 + ci
                        gt = state["tiles"][int(cgrp2[c])]
                        nc.tensor.matmul(
                            agg2[:, q, :], ohc2[:, c, :],
                            gt[:, int(coff2[c]), :],
                            start=(ci == 0), stop=(ci == cn - 1))
                    if q == 1:
                        st = stp.tile([P, 2, D_H1], f8, tag="st2")
                        nc.vector.tensor_copy(st[:], agg2[:])
                        nc.sync.dma_start(
                            p2[g - 1:g + 1].rearrange("g p f -> p g f"), st[:])

            if "noag" not in abl:
                act_reduce_scatter(p2.opt(), m2.opt())

            # ================= Layer 2 own blocks + t3 =================
            mtall = cp.tile([P, NB, D_H1], f8, tag="mtall")
            if "l2o" not in abl:
                nc.sync.dma_start(mtall[:, 0:2, :],
                                  m2[0:2].rearrange("j p f -> p j f"))
                nc.sync.dma_start(mtall[:, 2:NB, :],
                                  m2[2:NB].rearrange("j p f -> p j f"))
            meanT_all = cp.tile([P, 2, NB, P], bf16, tag="meanTall")
            NBo = NB if "l2o" not in abl else 0

            def transpose_mean(j):
                for k in range(2):
                    tpp = psA.tile([P, D_IN], f32, tag="agg1")
                    nc.tensor.matmul(tpp[:], mtall[:, j, k * P:(k + 1) * P],
                                     diag[:, j, :], start=True, stop=True)
                    nc.vector.tensor_copy(meanT_all[:, k, j, :], tpp[:])

            if NBo:
                transpose_mean(0)
                transpose_mean(1)
            for jp in range(NBo // 2):
                j0 = 2 * jp
                for jn in (2 * jp + 2, 2 * jp + 3):
                    if jn < NBo:
                        transpose_mean(jn)
                psl = slice(j0 * P, (j0 + 2) * P)
                hT = htp.tile([P, 8, 2 * P], bf16, tag="hT")
                for s in range(8):
                    hp = psO.tile([P, D_H1], f32, tag="outp")
                    sl = slice(s * P, (s + 1) * P)
                    nc.tensor.matmul(hp[:], w2l_sb[:, 0, sl],
                                     meanT_all[:, 0, j0:j0 + 2, :],
                                     start=True, stop=False)
                    nc.tensor.matmul(hp[:], w2l_sb[:, 1, sl],
                                     meanT_all[:, 1, j0:j0 + 2, :],
                                     start=False, stop=False)
                    nc.tensor.matmul(hp[:], w2r_sb[:, 0, sl],
                                     h1T_res[:, 0, psl], start=False,
                                     stop=False)
                    nc.tensor.matmul(hp[:], w2r_sb[:, 1, sl],
                                     h1T_res[:, 1, psl], start=False,
                                     stop=True)
                    nc.vector.tensor_scalar(hT[:, s, :], hp[:],
                                            b2t_sb[:, s:s + 1], 0.0, ADD, MAX)

                for j in (j0, j0 + 1):
                    half = slice((j - j0) * P, (j - j0 + 1) * P)
                    tr = psO.tile([P, D_H1], f32, tag="outp")
                    for s in range(8):
                        nc.tensor.matmul(tr[:, 0:P], hT[:, s, half],
                                         w3lr_sb[:, s, :],
                                         start=(s == 0), stop=False)
                    nc.tensor.matmul(tr[:, 0:P], ones_t[:], b3_sb[:],
                                     start=False, stop=True)
                    t3blk = smp.tile([P, P], bf16, tag="t3blk")
                    nc.vector.tensor_copy(t3blk[:, 0:D_OUT], tr[:, 0:D_OUT])
                    nc.vector.memset(t3blk[:, D_OUT:P], 0.0)
                    nc.vector.tensor_copy(r3_res[:, j, :], tr[:, D_OUT:P])
                    nc.sync.dma_start(t3_own[j * P:(j + 1) * P, :], t3blk[:])

            # ========== Layer 3 partial sums for ALL dst blocks ==========
            if "l3p" not in abl:
                state["next"] = 0
                state["tiles"] = {}
                agg3 = None
                for g in range(NBLK):
                    c0, cn = cstart[g], int(C2[g])
                    need_chunks(gp3, t3_own[:], P, "gath3",
                                min(c0 + cn + 32, NCH2), groups3, GSZ3)
                    q = g % 8
                    if q == 0:
                        agg3 = psA.tile([P, 2, D_H1], f32, tag="agg2")
                    for ci in range(cn):
                        c = c0 + ci
                        gt = state["tiles"][int(cgrp3[c])]
                        nc.tensor.matmul(
                            agg3[:, q // 4, (q % 4) * D_OUT:
                                 (q % 4 + 1) * D_OUT], ohc2[:, c, :],
                            gt[:, int(coff3[c]), 0:D_OUT],
                            start=(ci == 0), stop=(ci == cn - 1))
                    if q == 7:
                        st = stp.tile([P, 8 * D_OUT], bf16, tag="st3")
                        nc.vector.tensor_copy(
                            st[:], agg3[:].rearrange("p a b -> p (a b)"))
                        nc.sync.dma_start(
                            p3[g - 7:g + 1].rearrange("g p f -> p g f"), st[:])

            if "noag" not in abl:
                act_reduce_scatter(p3.opt(), m3.opt())

            # ============ Layer 3 own rows + log_softmax ============
            m3all = cp.tile([P, NB, D_OUT], bf16, tag="m3all")
            if "l3o" not in abl:
                nc.sync.dma_start(m3all[:, 0:NB // 2, :],
                                  m3[0:NB // 2].rearrange("j p f -> p j f"))
                nc.sync.dma_start(m3all[:, NB // 2:NB, :],
                                  m3[NB // 2:NB].rearrange("j p f -> p j f"))
            nc.vector.memset(ssum_res[:], 1.0)
            for j in range(NB if "l3o" not in abl else 0):
                nc.vector.scalar_tensor_tensor(
                    ob_res[:, j, :], m3all[:, j, :], invd_sb[:, j:j + 1],
                    r3_res[:, j, :], MUL, ADD)
                # |logits| is O(10): exp fits f32 comfortably, so skip the
                # max-subtraction and its reduce on the critical chain
                e = smp.tile([P, D_OUT], f32, tag="e")
                nc.scalar.activation(e[:], ob_res[:, j, :], EXP)
                nc.vector.tensor_reduce(ssum_res[:, j:j + 1], e[:], AXX, ADD)
            if "l3o" not in abl:
                # out = y - ln(sum(exp(y)))
                nc.scalar.activation(ls_res[:], ssum_res[:], LN)
                for j in range(NB):
                    nc.vector.tensor_scalar(obf_res[:, j, :], ob_res[:, j, :],
                                            ls_res[:, j:j + 1], None, SUB)
                    if j == NB // 2 - 1:
                        nc.sync.dma_start(
                            outp[0:NB // 2 * P].rearrange(
                                "(j p) f -> p j f", p=P),
                            obf_res[:, 0:NB // 2, :])
                nc.sync.dma_start(
                    outp[NB // 2 * P:].rearrange("(j p) f -> p j f", p=P),
                    obf_res[:, NB // 2:NB, :])

    nc.compile()
    return nc


def _wrap16(a):
    """idx i -> partition i%16, col i//16; replicated to 128 partitions."""
    w = a.reshape(-1, 16).T
    return np.ascontiguousarray(np.tile(w, (8, 1)))


def _pack_oh(acc, nch):
    """[nch*P, P] f32 multi-hot matrix -> [P, nch*P] fp8 SBUF layout.

    SBUF layout: partition = slot%128 (slot within chunk), free dims =
    (chunk, dst-local). Values are edge multiplicities (0/1/2...).
    """
    oh = acc.reshape(nch, P, P).transpose(1, 0, 2).reshape(P, nch * P)
    return np.ascontiguousarray(oh).astype(BF)


def _balanced_perm(deg, seed=0):
    """Assign nodes to 80 blocks of 128 so block in-degree sums are even."""
    import heapq
    nblk = NPAD // P
    if seed:
        rng = np.random.RandomState(seed)
        order = np.argsort(-deg + rng.uniform(0, 0.5, deg.shape),
                           kind="stable")
    else:
        order = np.argsort(-deg, kind="stable")
    heap = [(0, 0, g) for g in range(nblk)]
    heapq.heapify(heap)
    newpos = np.empty(NPAD, np.int64)
    fill = np.zeros(nblk, np.int64)
    for n in order:
        s, _, g = heapq.heappop(heap)
        newpos[n] = g * P + fill[g]
        fill[g] += 1
        if fill[g] < P:
            heapq.heappush(heap, (s + int(deg[n]), int(fill[g]), g))
    return newpos


def _best_perm(deg, psrc_of, pdst_of):
    """Pick the tie-break seed minimizing total L2/L3 chunk count."""
    best = None
    for seed in range(24):
        newpos = _balanced_perm(deg, seed)
        psrc, pdst = psrc_of(newpos), pdst_of(newpos)
        # distinct (src, dst-block) pairs per cell, padded to chunks of 128
        key = psrc * np.int64(NBLK) + pdst // P
        uniq = np.unique(key)
        ucell = np.bincount((uniq // NBLK // PER_CORE) * NBLK
                            + uniq % NBLK, minlength=NCORES * NBLK)
        ucell = ucell.reshape(NCORES, NBLK)
        nch = int(np.maximum(1, np.ceil(ucell.max(axis=0) / P)).sum())
        # distinct in-edge sources per dst block (sets MC1 / L1 rows)
        ikey = pdst * np.int64(NPAD) + psrc
        iu = np.unique(ikey)
        iucnt = np.bincount(iu // NPAD // P, minlength=NBLK)
        mc1 = int(np.ceil(iucnt.max() / P))
        cost = 2.82 * nch + 9.9 * mc1
        if best is None or cost < best[0]:
            best = (cost, newpos)
    return best[1]


def _prep(x, edge_index):
    src = np.asarray(edge_index[0], dtype=np.int64)
    dst = np.asarray(edge_index[1], dtype=np.int64)
    deg = np.bincount(dst, minlength=NPAD).astype(np.float64)
    invdeg_n = (1.0 / np.maximum(deg, 1.0)).astype(np.float32)

    newpos = _best_perm(deg, lambda np_: np_[src], lambda np_: np_[dst])
    oldnode = np.empty(NPAD, np.int64)
    oldnode[newpos] = np.arange(NPAD)
    psrc = newpos[src]
    pdst = newpos[dst]

    # ---------- L1: in-edges grouped by own dst block (dedup by src) ----
    order = np.argsort(pdst, kind="stable")
    dsts = pdst[order]
    srcs = psrc[order]
    starts = np.searchsorted(dsts, np.arange(0, NPAD + P, P))
    l1 = []
    for g in range(NBLK):
        lo, hi = int(starts[g]), int(starts[g + 1])
        u, inv = np.unique(srcs[lo:hi], return_inverse=True)
        l1.append((u, inv, dsts[lo:hi] - g * P))
    MC1 = max(1, int(np.ceil(max(len(u) for u, _, _ in l1) / P)))

    xp = np.zeros((NPAD, D_IN), dtype=np.float32)
    xp[:N_NODES] = x
    xp = xp[oldnode]           # permuted node order

    per_core_l1 = []
    for r in range(NCORES):
        sg = np.zeros(NB * MC1 * P, np.int16)
        ohacc = np.zeros((NB * MC1 * P, P), np.float32)
        for j in range(NB):
            u, inv, dl = l1[r * NB + j]
            base = j * MC1 * P
            sg[base:base + len(u)] = u.astype(np.int16)
            np.add.at(ohacc, (base + inv, dl), 1.0)
        per_core_l1.append((_wrap16(sg), _pack_oh(ohacc, NB * MC1)))

    # ---------- L2/L3 partial: out-edges by (src core, dst blk), dedup ----
    srcc = psrc // PER_CORE
    l2meta = [[None] * NBLK for _ in range(NCORES)]
    ucnt = np.zeros((NCORES, NBLK), np.int64)
    for r in range(NCORES):
        mask = srcc == r
        es, ed = psrc[mask], pdst[mask]
        o = np.argsort(ed, kind="stable")
        es, ed = es[o], ed[o]
        bst = np.searchsorted(ed, np.arange(0, NPAD + P, P))
        for g in range(NBLK):
            lo, hi = int(bst[g]), int(bst[g + 1])
            u, inv = np.unique(es[lo:hi], return_inverse=True)
            l2meta[r][g] = (u, inv, ed[lo:hi] - g * P)
            ucnt[r, g] = len(u)
    C2 = np.maximum(1, np.ceil(ucnt.max(axis=0) / P)).astype(np.int64)
    cstart = np.concatenate([[0], np.cumsum(C2)])
    NCH2 = int(cstart[-1])

    per_core_l2 = []
    for r in range(NCORES):
        sg = np.zeros(NCH2 * P, np.int16)
        ohacc = np.zeros((NCH2 * P, P), np.float32)
        for g in range(NBLK):
            u, inv, dl = l2meta[r][g]
            base = int(cstart[g]) * P
            sg[base:base + len(u)] = (u - r * PER_CORE).astype(np.int16)
            np.add.at(ohacc, (base + inv, dl), 1.0)
        per_core_l2.append((_wrap16(sg), _pack_oh(ohacc, NCH2)))

    # per-core inverse degree of own nodes: [P, NB] (partition p, block j)
    invd_own = invdeg_n[oldnode].reshape(NCORES, NB, P)
    invd_own = [np.ascontiguousarray(invd_own[r].T) for r in range(NCORES)]

    return xp, per_core_l1, MC1, per_core_l2, tuple(int(c) for c in C2), \
        invd_own, newpos


def _make_in_maps(x, edge_index, w1l, w1r, b1, w2l, w2r, b2, w3l, w3r, b3):
    import ml_dtypes
    x = np.ascontiguousarray(np.asarray(x, dtype=np.float32))
    xp, pc1, MC1, pc2, C2, invd_own, newpos = _prep(x, np.asarray(edge_index))

    b1v = np.asarray(b1, np.float32).reshape(-1)
    b2v = np.asarray(b2, np.float32).reshape(-1)
    xbf = xp.astype(BF)
    common = {
        "xbf": xbf,
        "w1l": np.asarray(w1l, np.float32).astype(BF),
        "w1r": np.asarray(w1r, np.float32).astype(BF),
        "b1": b1v.reshape(1, D_H1).astype(BF),
        "b1t": np.ascontiguousarray(b1v.reshape(2, P).T),
        "w2l": np.asarray(w2l, np.float32).astype(BF),
        "w2r": np.asarray(w2r, np.float32).astype(BF),
        "b2t": np.ascontiguousarray(b2v.reshape(8, P).T),
        "w3lr": np.ascontiguousarray(np.concatenate(
            [np.asarray(w3l, np.float32), np.asarray(w3r, np.float32)],
            axis=1)).astype(BF),
        "b3pad": np.concatenate(
            [np.zeros(D_OUT, np.float32),
             np.asarray(b3, np.float32).reshape(-1)]).reshape(1, P).astype(BF),
        "ident_in": np.eye(P, dtype=BF),
    }
    in_maps = []
    for r in range(NCORES):
        g1, o1 = pc1[r]
        g2, o2 = pc2[r]
        m = dict(common)
        m["xownT"] = np.ascontiguousarray(
            xbf[r * PER_CORE:(r + 1) * PER_CORE].T)
        m["gidx1"] = g1
        m["oh1_in"] = o1
        m["gidx2"] = g2
        m["oh2_in"] = o2
        m["invd_own"] = invd_own[r]
        dg = np.zeros((P, NB * P), np.float32)
        for j in range(NB):
            np.fill_diagonal(dg[:, j * P:(j + 1) * P], invd_own[r][:, j])
        m["diag_in"] = dg.astype(BF)
        in_maps.append(m)
    return in_maps, (MC1, C2), newpos


def kernel(x, edge_index, w1l, w1r, b1, w2l, w2r, b2, w3l, w3r, b3):
    global LAST_RESULTS
    import os
    from concourse.bass_utils import run_bass_kernel_spmd

    if os.environ.get("BASS_TRACE"):
        try:
            import antenv.axon_hooks  # noqa: F401
        except ImportError:
            os.environ.pop("BASS_TRACE", None)  # no NTFF hook here

    in_maps, key, newpos = _make_in_maps(x, edge_index, w1l, w1r, b1, w2l,
                                         w2r, b2, w3l, w3r, b3)
    if key not in _CACHE:
        _CACHE[key] = _build(*key)
    nc = _CACHE[key]

    res = run_bass_kernel_spmd(nc, in_maps, core_ids=list(range(NCORES)))
    LAST_RESULTS = res
    out = np.concatenate([res.results[r]["out"] for r in range(NCORES)], axis=0)
    return np.ascontiguousarray(out[newpos[:N_NODES]])
